# revision 6
# baseline (speedup 1.0000x reference)
"""Trainium2 Bass kernel for nn_EngramMemory_81415400063490 (embedding_lookup).

Contract: kernel(**inputs) takes the FULL unsharded inputs (numpy arrays, keyed
as in reference.setup_inputs()) and returns the FULL [4, 4096, 1024] float32
output. Internally shards data-parallel over the 8 NeuronCores (2048 tokens per
core + 128-token halo each side for the depthwise conv), replicates the hash
embedding tables + weights, runs one SPMD Bass program via
run_bass_kernel_spmd, and reassembles.
"""

import sys

sys.path.insert(0, "/opt/trn_rl_repo")

import numpy as np
import ml_dtypes

import concourse.bass as bass
import concourse.tile as tile
from concourse import bacc, mybir
from concourse.bass_utils import run_bass_kernel_spmd
from concourse.masks import make_identity

BF16 = ml_dtypes.bfloat16
AF = mybir.ActivationFunctionType
ALU = mybir.AluOpType

B, S, D = 4, 4096, 1024
VOCAB, HASH2, HASH3 = 50257, 10000, 50000
MULT = 2654435761
EPS = 1.1920928955078125e-07  # torch float32 eps, used by the RMSNorm
N_CORES = 8
T_CORE = (B * S) // N_CORES  # 2048 tokens per core
HALO = 128
T_EXT = T_CORE + 2 * HALO  # 2304 tokens incl. halos
NT = 256  # token tile size
NTILES = T_EXT // NT  # 9
DC = D // 128  # 8 feature chunks of 128
KC = (2 * D) // 128  # 16 contraction chunks for We
TT_OUT = T_CORE // 128  # 16 output token tiles

_PROG_CACHE = {}


def _build_program(with_web, with_wkb, with_wvb, with_convb):
    f32, bf16, i16 = mybir.dt.float32, mybir.dt.bfloat16, mybir.dt.int16
    nc = bacc.Bacc("TRN2", target_bir_lowering=False)

    emb2 = nc.dram_tensor("emb2", [HASH2, D], bf16, kind="ExternalInput")
    emb3p = nc.dram_tensor("emb3p", [HASH3 // 2, 2 * D], bf16, kind="ExternalInput")
    wet = nc.dram_tensor("wet", [2 * D, D], bf16, kind="ExternalInput")
    wkt = nc.dram_tensor("wkt", [D, D], bf16, kind="ExternalInput")
    wvt = nc.dram_tensor("wvt", [D, D], bf16, kind="ExternalInput")
    convw = nc.dram_tensor("convw", [128, DC, 3], f32, kind="ExternalInput")
    idx2r = nc.dram_tensor("idx2r", [128, T_EXT // 16], i16, kind="ExternalInput")
    idx3r = nc.dram_tensor("idx3r", [128, T_EXT // 16], i16, kind="ExternalInput")
    parity = nc.dram_tensor("parity", [1, T_EXT], mybir.dt.uint8, kind="ExternalInput")
    ymaskd = nc.dram_tensor("ymask", [1, T_EXT], f32, kind="ExternalInput")
    shsd = nc.dram_tensor("shs", [1, T_EXT], f32, kind="ExternalInput")
    hst = nc.dram_tensor("hst", [D, T_EXT], bf16, kind="ExternalInput")
    hsc = nc.dram_tensor("hsc", [T_CORE, D], f32, kind="ExternalInput")
    outp = nc.dram_tensor("outp", [T_CORE, D], f32, kind="ExternalOutput")
    web = wkb = wvb = convb = None
    if with_web:
        web = nc.dram_tensor("web", [1, D], bf16, kind="ExternalInput")
    if with_wkb:
        wkb = nc.dram_tensor("wkb", [1, D], bf16, kind="ExternalInput")
    if with_wvb:
        wvb = nc.dram_tensor("wvb", [1, D], bf16, kind="ExternalInput")
    if with_convb:
        convb = nc.dram_tensor("convb", [1, D], bf16, kind="ExternalInput")

    wet_r = wet.ap().rearrange("(k p) m -> p k m", p=128)  # [128, 16, 1024]
    wkt_r = wkt.ap().rearrange("(k p) m -> p k m", p=128)  # [128, 8, 1024]
    wvt_r = wvt.ap().rearrange("(k p) m -> p k m", p=128)
    hst_r = hst.ap().rearrange("(c p) t -> p c t", p=128)  # [128, 8, 2304]

    with tile.TileContext(nc) as tc:
        with tc.tile_pool(name="singles", bufs=1) as singles:
            wet_sb = singles.tile([128, KC, D], bf16)
            nc.sync.dma_start(out=wet_sb[:], in_=wet_r)
            wkt_sb = singles.tile([128, DC, D], bf16)
            nc.sync.dma_start(out=wkt_sb[:], in_=wkt_r)
            wvt_sb = singles.tile([128, DC, D], bf16)
            nc.sync.dma_start(out=wvt_sb[:], in_=wvt_r)
            convw_sb = singles.tile([128, DC, 3], f32)
            nc.sync.dma_start(out=convw_sb[:], in_=convw.ap())
            idx2_sb = singles.tile([128, T_EXT // 16], i16)
            nc.sync.dma_start(out=idx2_sb[:], in_=idx2r.ap())
            idx3_sb = singles.tile([128, T_EXT // 16], i16)
            nc.sync.dma_start(out=idx3_sb[:], in_=idx3r.ap())
            # parity mask broadcast to all 128 partitions (DMA partition-bcast)
            par_sb = singles.tile([128, T_EXT], mybir.dt.uint8)
            par_bcast = bass.AP(
                tensor=parity.ap().tensor,
                offset=0,
                ap=[[0, 128], [1, T_EXT]],
            )
            nc.gpsimd.dma_start(out=par_sb[:], in_=par_bcast)
            ymask_sb = singles.tile([1, T_EXT], f32)
            nc.sync.dma_start(out=ymask_sb[:], in_=ymaskd.ap())
            shs_sb = singles.tile([1, T_EXT], f32)
            nc.sync.dma_start(out=shs_sb[:], in_=shsd.ap())
            ones_col_bf = singles.tile([128, 1], bf16)
            nc.vector.memset(ones_col_bf[:], 1.0)
            ones_row_f = singles.tile([1, 128], f32)
            nc.vector.memset(ones_row_f[:], 1.0)
            identity_bf = singles.tile([128, 128], bf16)
            make_identity(nc, identity_bf[:])
            ones_nt_bf = singles.tile([1, NT], bf16)
            nc.vector.memset(ones_nt_bf[:], 1.0)
            eps_sb = singles.tile([1, 1], mybir.dt.float32)
            nc.vector.memset(eps_sb[:], float(EPS))
            bias_sbs = {}
            for name, t in (("web", web), ("wkb", wkb), ("wvb", wvb)):
                if t is not None:
                    bsb = singles.tile([1, D], bf16)
                    nc.sync.dma_start(out=bsb[:], in_=t.ap())
                    bias_sbs[name] = bsb
            if convb is not None:
                convb_sb = singles.tile([1, D], bf16)
                nc.sync.dma_start(out=convb_sb[:], in_=convb.ap())
                ones_row_bf = singles.tile([1, 128], bf16)
                nc.vector.memset(ones_row_bf[:], 1.0)
            # resident y (alpha * v_e), feature-major, bf16
            y_sb = singles.tile([128, DC, T_EXT], bf16)

            # ---------------- phase 1: per-tile main pipeline ----------------
            with (
                tc.tile_pool(name="g2", bufs=2) as g2p,
                tc.tile_pool(name="g3", bufs=2) as g3p,
                tc.tile_pool(name="hstp", bufs=2) as hstp,
                tc.tile_pool(name="work", bufs=2) as work,
                tc.tile_pool(name="small", bufs=2) as small,
                tc.tile_pool(name="psb", bufs=3, space="PSUM") as psum_big,
                tc.tile_pool(name="pss", bufs=2, space="PSUM") as psum_small,
            ):
                for i in range(NTILES):
                    t0 = i * NT
                    e2 = g2p.tile([128, DC, NT], bf16, tag="e2")
                    nc.gpsimd.dma_gather(
                        out_ap=e2[:],
                        in_ap=emb2.ap(),
                        idxs_ap=idx2_sb[:, i * (NT // 16) : (i + 1) * (NT // 16)],
                        num_idxs=NT,
                        num_idxs_reg=NT,
                        elem_size=D,
                        transpose=True,
                    )
                    e3r = g3p.tile([128, 2 * DC, NT], bf16, tag="e3r")
                    nc.gpsimd.dma_gather(
                        out_ap=e3r[:],
                        in_ap=emb3p.ap(),
                        idxs_ap=idx3_sb[:, i * (NT // 16) : (i + 1) * (NT // 16)],
                        num_idxs=NT,
                        num_idxs_reg=NT,
                        elem_size=2 * D,
                        transpose=True,
                    )
                    # select odd rows where parity==1 (overwrite even half)
                    par_slice = par_sb[:, t0 : t0 + NT]
                    for cc in range(DC):
                        nc.vector.copy_predicated(
                            out=e3r[:, cc, :],
                            mask=par_slice,
                            data=e3r[:, DC + cc, :],
                        )

                    hst_t = hstp.tile([128, DC, NT], bf16, tag="hst")
                    nc.sync.dma_start(out=hst_t[:], in_=hst_r[:, :, t0 : t0 + NT])

                    et = work.tile([128, DC, NT], bf16, tag="et")
                    et2 = work.tile([128, DC, NT], bf16, tag="et2")
                    for m in range(DC):
                        pet = psum_big.tile([128, NT], mybir.dt.float32, tag="pbig")
                        for k in range(KC):
                            rhs = e2[:, k, :] if k < DC else e3r[:, k - DC, :]
                            nc.tensor.matmul(
                                pet[:],
                                wet_sb[:, k, m * 128 : (m + 1) * 128],
                                rhs,
                                start=(k == 0),
                                stop=(k == KC - 1 and web is None),
                            )
                        if web is not None:
                            nc.tensor.matmul(
                                pet[:],
                                bias_sbs["web"][:, m * 128 : (m + 1) * 128],
                                ones_nt_bf[:],
                                start=False,
                                stop=True,
                            )
                        nc.scalar.activation(et[:, m, :], pet[:], AF.Copy)
                        nc.vector.tensor_mul(et2[:, m, :], et[:, m, :], et[:, m, :])
                    # mean-square over features via ones-matmul partition reduce
                    pms = psum_small.tile([1, NT], mybir.dt.float32, tag="psmall1")
                    for m in range(DC):
                        nc.tensor.matmul(
                            pms[:],
                            ones_col_bf[:],
                            et2[:, m, :],
                            start=(m == 0),
                            stop=(m == DC - 1),
                        )
                    sq = small.tile([1, NT], mybir.dt.float32, tag="sq")
                    nc.scalar.activation(
                        sq[:], pms[:], AF.Sqrt, bias=eps_sb[:], scale=1.0 / D
                    )
                    se = small.tile([1, NT], mybir.dt.float32, tag="se")
                    nc.vector.reciprocal(se[:], sq[:])
                    psb_t = psum_small.tile([128, NT], mybir.dt.float32, tag="psmall2")
                    nc.tensor.matmul(psb_t[:], ones_row_f[:], se[:], start=True, stop=True)
                    sbf = work.tile([128, NT], bf16, tag="sbf")
                    nc.scalar.activation(sbf[:], psb_t[:], AF.Copy)
                    en = work.tile([128, DC, NT], bf16, tag="en")
                    for m in range(DC):
                        nc.vector.tensor_mul(en[:, m, :], et[:, m, :], sbf[:])
                    # k_e = WkT' @ e_norm ; prod = k_e * h (norm_w folded on host)
                    prod = work.tile([128, DC, NT], bf16, tag="prod")
                    for m in range(DC):
                        pke = psum_big.tile([128, NT], mybir.dt.float32, tag="pbig")
                        for k in range(DC):
                            nc.tensor.matmul(
                                pke[:],
                                wkt_sb[:, k, m * 128 : (m + 1) * 128],
                                en[:, k, :],
                                start=(k == 0),
                                stop=(k == DC - 1 and wkb is None),
                            )
                        if wkb is not None:
                            nc.tensor.matmul(
                                pke[:],
                                bias_sbs["wkb"][:, m * 128 : (m + 1) * 128],
                                ones_nt_bf[:],
                                start=False,
                                stop=True,
                            )
                        nc.vector.tensor_mul(prod[:, m, :], pke[:], hst_t[:, m, :])
                    pdot = psum_small.tile([1, NT], mybir.dt.float32, tag="psmall1")
                    for m in range(DC):
                        nc.tensor.matmul(
                            pdot[:],
                            ones_col_bf[:],
                            prod[:, m, :],
                            start=(m == 0),
                            stop=(m == DC - 1),
                        )
                    d2 = small.tile([1, NT], mybir.dt.float32, tag="d2")
                    nc.vector.tensor_mul(d2[:], pdot[:], shs_sb[:, t0 : t0 + NT])
                    alph = small.tile([1, NT], mybir.dt.float32, tag="alph")
                    nc.scalar.activation(alph[:], d2[:], AF.Sigmoid)
                    alphm = small.tile([1, NT], mybir.dt.float32, tag="alphm")
                    nc.vector.tensor_mul(alphm[:], alph[:], ymask_sb[:, t0 : t0 + NT])
                    pab = psum_small.tile([128, NT], mybir.dt.float32, tag="psmall2")
                    nc.tensor.matmul(pab[:], ones_row_f[:], alphm[:], start=True, stop=True)
                    abf = work.tile([128, NT], bf16, tag="abf")
                    nc.scalar.activation(abf[:], pab[:], AF.Copy)
                    # v_e = WvT @ e_t ; y = alpha * v_e  (into resident y)
                    for m in range(DC):
                        pve = psum_big.tile([128, NT], mybir.dt.float32, tag="pbig")
                        for k in range(DC):
                            nc.tensor.matmul(
                                pve[:],
                                wvt_sb[:, k, m * 128 : (m + 1) * 128],
                                et[:, k, :],
                                start=(k == 0),
                                stop=(k == DC - 1 and wvb is None),
                            )
                        if wvb is not None:
                            nc.tensor.matmul(
                                pve[:],
                                bias_sbs["wvb"][:, m * 128 : (m + 1) * 128],
                                ones_nt_bf[:],
                                start=False,
                                stop=True,
                            )
                        nc.vector.tensor_mul(y_sb[:, m, t0 : t0 + NT], pve[:], abf[:])

            # ---------------- phase 2: depthwise conv along tokens ----------
            with tc.tile_pool(name="convp", bufs=1) as convp:
                u_sb = convp.tile([128, DC, T_CORE], bf16)
                for c in range(DC):
                    nc.vector.tensor_scalar(
                        out=u_sb[:, c, :],
                        in0=y_sb[:, c, HALO - 1 : HALO - 1 + T_CORE],
                        scalar1=convw_sb[:, c, 0:1],
                        scalar2=None,
                        op0=ALU.mult,
                    )
                    nc.vector.scalar_tensor_tensor(
                        out=u_sb[:, c, :],
                        in0=y_sb[:, c, HALO : HALO + T_CORE],
                        scalar=convw_sb[:, c, 1:2],
                        in1=u_sb[:, c, :],
                        op0=ALU.mult,
                        op1=ALU.add,
                    )
                    nc.vector.scalar_tensor_tensor(
                        out=u_sb[:, c, :],
                        in0=y_sb[:, c, HALO + 1 : HALO + 1 + T_CORE],
                        scalar=convw_sb[:, c, 2:3],
                        in1=u_sb[:, c, :],
                        op0=ALU.mult,
                        op1=ALU.add,
                    )

                # ------------- phase 3: transpose back + residual add -------
                with (
                    tc.tile_pool(name="outs", bufs=3) as outs,
                    tc.tile_pool(name="pso", bufs=2, space="PSUM") as psum_out,
                ):
                    for tt in range(TT_OUT):
                        pu = psum_out.tile([128, D], bf16, tag="pu")
                        if convb is not None:
                            for half in range(2):
                                nc.tensor.matmul(
                                    pu[:, half * 512 : (half + 1) * 512],
                                    ones_row_bf[:],
                                    convb_sb[:, half * 512 : (half + 1) * 512],
                                    start=True,
                                    stop=False,
                                )
                        for c in range(DC):
                            nc.tensor.matmul(
                                pu[:, c * 128 : (c + 1) * 128],
                                u_sb[:, c, tt * 128 : (tt + 1) * 128],
                                identity_bf[:],
                                is_transpose=True,
                                start=(convb is None),
                                stop=True,
                            )
                        hs_t = outs.tile([128, D], mybir.dt.float32, tag="hs")
                        nc.sync.dma_start(
                            out=hs_t[:], in_=hsc.ap()[tt * 128 : (tt + 1) * 128, :]
                        )
                        ot = outs.tile([128, D], mybir.dt.float32, tag="ot")
                        nc.vector.tensor_add(ot[:], pu[:], hs_t[:])
                        nc.sync.dma_start(
                            out=outp.ap()[tt * 128 : (tt + 1) * 128, :], in_=ot[:]
                        )

    nc.compile()
    return nc


def _get_program(flags):
    if flags not in _PROG_CACHE:
        _PROG_CACHE[flags] = _build_program(*flags)
    return _PROG_CACHE[flags]


def _host_prep(inputs):
    hs = np.asarray(inputs["hidden_states"], dtype=np.float32)
    ids = np.asarray(inputs["input_ids"], dtype=np.int64)
    vproj = np.asarray(inputs["vocab_projection"], dtype=np.int64)
    emb2 = np.asarray(inputs["emb2"], dtype=np.float32)
    emb3 = np.asarray(inputs["emb3"], dtype=np.float32)
    We_w = np.asarray(inputs["We_w"], dtype=np.float32)
    We_b = np.asarray(inputs["We_b"], dtype=np.float32)
    Wv_w = np.asarray(inputs["Wv_w"], dtype=np.float32)
    Wv_b = np.asarray(inputs["Wv_b"], dtype=np.float32)
    Wk_w = np.asarray(inputs["Wk_w"], dtype=np.float32)
    Wk_b = np.asarray(inputs["Wk_b"], dtype=np.float32)
    conv_w = np.asarray(inputs["conv_w"], dtype=np.float32)
    conv_b = np.asarray(inputs["conv_b"], dtype=np.float32)
    norm_w = np.asarray(inputs["norm_w"], dtype=np.float32)

    # exact integer hash indices (host, int64)
    comp = vproj[ids]  # [B, S]
    padded = np.pad(comp, ((0, 0), (2, 0)))
    bi = padded[:, 0:S] + padded[:, 1 : S + 1]
    tri = bi + padded[:, 2 : S + 2]
    idx2 = ((bi * MULT) % HASH2).reshape(-1)
    idx3 = ((tri * MULT) % HASH3).reshape(-1)

    hsf = hs.reshape(B * S, D)
    msh = np.mean(np.square(hsf.astype(np.float64)), axis=1)
    sh = (1.0 / (np.sqrt(msh + EPS) * np.sqrt(D))).astype(np.float32)  # [B*S]
    hsn = hsf * norm_w[None, :]

    shared = {
        "emb2": emb2.astype(BF16),
        "emb3p": emb3.astype(BF16).reshape(HASH3 // 2, 2 * D),
        "wet": np.ascontiguousarray(We_w.T).astype(BF16),
        "wkt": np.ascontiguousarray(norm_w[:, None] * Wk_w.T).astype(BF16),
        "wvt": np.ascontiguousarray(Wv_w.T).astype(BF16),
        "convw": np.ascontiguousarray(
            conv_w[:, 0, :].reshape(DC, 128, 3).transpose(1, 0, 2)
        ).astype(np.float32),
    }
    flags = (
        bool(np.any(We_b)),
        bool(np.any(Wk_b)),
        bool(np.any(Wv_b)),
        bool(np.any(conv_b)),
    )
    if flags[0]:
        shared["web"] = We_b.reshape(1, D).astype(BF16)
    if flags[1]:
        shared["wkb"] = Wk_b.reshape(1, D).astype(BF16)
    if flags[2]:
        shared["wvb"] = Wv_b.reshape(1, D).astype(BF16)
    if flags[3]:
        shared["convb"] = conv_b.reshape(1, D).astype(BF16)

    def wrap16(a):
        return np.ascontiguousarray(
            np.tile(a.astype(np.int16).reshape(T_EXT // 16, 16).T, (8, 1))
        )

    in_maps = []
    for c in range(N_CORES):
        s0 = c * T_CORE
        ext = np.arange(s0 - HALO, s0 + T_CORE + HALO)
        cl = np.clip(ext, 0, B * S - 1)
        row = s0 // S
        inrow = ((ext >= row * S) & (ext < (row + 1) * S)).astype(np.float32)
        i2e = idx2[cl]
        i3e = idx3[cl]
        m = dict(shared)
        m["idx2r"] = wrap16(i2e)
        m["idx3r"] = wrap16(i3e >> 1)
        m["parity"] = (i3e & 1).astype(np.uint8)[None, :]
        m["ymask"] = np.ascontiguousarray(inrow[None, :])
        m["shs"] = np.ascontiguousarray(sh[cl][None, :])
        m["hst"] = np.ascontiguousarray(hsn[cl].T).astype(BF16)
        m["hsc"] = np.ascontiguousarray(hsf[s0 : s0 + T_CORE])
        in_maps.append(m)
    return flags, in_maps


def kernel(**inputs) -> np.ndarray:
    flags, in_maps = _host_prep(inputs)
    nc = _get_program(flags)
    res = run_bass_kernel_spmd(nc, in_maps, core_ids=list(range(N_CORES)))
    out = np.concatenate(
        [res.results[c]["outp"] for c in range(N_CORES)], axis=0
    ).reshape(B, S, D)
    return np.ascontiguousarray(out, dtype=np.float32)


# revision 7
# speedup vs baseline: 1.1554x; 1.1554x over previous
"""Trainium2 Bass kernel for nn_EngramMemory_81415400063490 (embedding_lookup).

Contract: kernel(**inputs) takes the FULL unsharded inputs (numpy arrays, keyed
as in reference.setup_inputs()) and returns the FULL [4, 4096, 1024] float32
output. Internally shards data-parallel over the 8 NeuronCores (2048 tokens per
core + 128-token halo each side for the depthwise conv), replicates the hash
embedding tables + weights, runs one SPMD Bass program via
run_bass_kernel_spmd, and reassembles.

Device dataflow per core (feature-major activations, bf16 matmuls):
  dma_gather(transpose=True) pulls emb2 rows and emb3 row-PAIRS (the pair
  trick keeps indices inside int16) straight into feature-major layout; a
  predicated copy selects the odd row where idx3 is odd. A lag-1 software
  pipeline overlaps tile i+1's gather + We matmuls with tile i's dependent
  chain (RMS scale, Wk/dot/sigmoid, Wv, y=alpha*v) and tile i-1's conv +
  PE-transpose + residual-add + store.
"""

import sys

sys.path.insert(0, "/opt/trn_rl_repo")

import numpy as np
import ml_dtypes

import concourse.bass as bass
import concourse.tile as tile
from concourse import bacc, mybir
from concourse.bass_utils import run_bass_kernel_spmd
from concourse.masks import make_identity

BF16 = ml_dtypes.bfloat16
AF = mybir.ActivationFunctionType
ALU = mybir.AluOpType

B, S, D = 4, 4096, 1024
VOCAB, HASH2, HASH3 = 50257, 10000, 50000
MULT = 2654435761
EPS = 1.1920928955078125e-07  # torch float32 eps, used by the RMSNorm
N_CORES = 8
T_CORE = (B * S) // N_CORES  # 2048 tokens per core
HALO = 128
T_EXT = T_CORE + 2 * HALO  # 2304 tokens incl. halos
NT = 256  # token tile size
NTILES = T_EXT // NT  # 9
DC = D // 128  # 8 feature chunks of 128
KC = (2 * D) // 128  # 16 contraction chunks for We

_PROG_CACHE = {}


def _build_program(with_web, with_wkb, with_wvb, with_convb):
    f32, bf16, i16 = mybir.dt.float32, mybir.dt.bfloat16, mybir.dt.int16
    nc = bacc.Bacc("TRN2", target_bir_lowering=False)

    emb2 = nc.dram_tensor("emb2", [HASH2, D], bf16, kind="ExternalInput")
    emb3p = nc.dram_tensor("emb3p", [HASH3 // 2, 2 * D], bf16, kind="ExternalInput")
    wet = nc.dram_tensor("wet", [2 * D, D], bf16, kind="ExternalInput")
    wkt = nc.dram_tensor("wkt", [D, D], bf16, kind="ExternalInput")
    wvt = nc.dram_tensor("wvt", [D, D], bf16, kind="ExternalInput")
    convw = nc.dram_tensor("convw", [128, DC, 3], f32, kind="ExternalInput")
    idx2r = nc.dram_tensor("idx2r", [128, T_EXT // 16], i16, kind="ExternalInput")
    idx3r = nc.dram_tensor("idx3r", [128, T_EXT // 16], i16, kind="ExternalInput")
    parity = nc.dram_tensor("parity", [1, T_EXT], mybir.dt.uint8, kind="ExternalInput")
    ymaskd = nc.dram_tensor("ymask", [1, T_EXT], bf16, kind="ExternalInput")
    shsd = nc.dram_tensor("shs", [1, T_EXT], f32, kind="ExternalInput")
    hst = nc.dram_tensor("hst", [D, T_EXT], bf16, kind="ExternalInput")
    hsc = nc.dram_tensor("hsc", [T_CORE, D], f32, kind="ExternalInput")
    outp = nc.dram_tensor("outp", [T_CORE, D], f32, kind="ExternalOutput")
    web = wkb = wvb = convb = None
    if with_web:
        web = nc.dram_tensor("web", [1, D], bf16, kind="ExternalInput")
    if with_wkb:
        wkb = nc.dram_tensor("wkb", [1, D], bf16, kind="ExternalInput")
    if with_wvb:
        wvb = nc.dram_tensor("wvb", [1, D], bf16, kind="ExternalInput")
    if with_convb:
        convb = nc.dram_tensor("convb", [1, D], bf16, kind="ExternalInput")

    wet_r = wet.ap().rearrange("(k p) m -> p k m", p=128)  # [128, 16, 1024]
    wkt_r = wkt.ap().rearrange("(k p) m -> p k m", p=128)  # [128, 8, 1024]
    wvt_r = wvt.ap().rearrange("(k p) m -> p k m", p=128)
    hst_r = hst.ap().rearrange("(c p) t -> p c t", p=128)  # [128, 8, 2304]

    import contextlib

    with tile.TileContext(nc) as tc, contextlib.ExitStack() as ctx:
        singles = ctx.enter_context(tc.tile_pool(name="singles", bufs=1))
        wet_sb = singles.tile([128, KC, D], bf16)
        nc.sync.dma_start(out=wet_sb[:], in_=wet_r)
        wkt_sb = singles.tile([128, DC, D], bf16)
        nc.sync.dma_start(out=wkt_sb[:], in_=wkt_r)
        wvt_sb = singles.tile([128, DC, D], bf16)
        nc.sync.dma_start(out=wvt_sb[:], in_=wvt_r)
        convw_sb = singles.tile([128, DC, 3], f32)
        nc.sync.dma_start(out=convw_sb[:], in_=convw.ap())
        idx2_sb = singles.tile([128, T_EXT // 16], i16)
        nc.sync.dma_start(out=idx2_sb[:], in_=idx2r.ap())
        idx3_sb = singles.tile([128, T_EXT // 16], i16)
        nc.sync.dma_start(out=idx3_sb[:], in_=idx3r.ap())
        par_sb = singles.tile([128, T_EXT], mybir.dt.uint8)
        par_bcast = bass.AP(
            tensor=parity.ap().tensor, offset=0, ap=[[0, 128], [1, T_EXT]]
        )
        nc.gpsimd.dma_start(out=par_sb[:], in_=par_bcast)
        ymask_sb = singles.tile([1, T_EXT], bf16)
        nc.sync.dma_start(out=ymask_sb[:], in_=ymaskd.ap())
        shs_sb = singles.tile([1, T_EXT], f32)
        nc.sync.dma_start(out=shs_sb[:], in_=shsd.ap())
        ones_col_bf = singles.tile([128, 1], bf16)
        nc.vector.memset(ones_col_bf[:], 1.0)
        ones_row_f = singles.tile([1, 128], f32)
        nc.vector.memset(ones_row_f[:], 1.0)
        identity_bf = singles.tile([128, 128], bf16)
        make_identity(nc, identity_bf[:])
        ones_nt_bf = singles.tile([1, NT], bf16)
        nc.vector.memset(ones_nt_bf[:], 1.0)
        eps_sb = singles.tile([1, 1], f32)
        nc.vector.memset(eps_sb[:], float(EPS))
        bias_sbs = {}
        for name, t in (("web", web), ("wkb", wkb), ("wvb", wvb)):
            if t is not None:
                bsb = singles.tile([1, D], bf16)
                nc.sync.dma_start(out=bsb[:], in_=t.ap())
                bias_sbs[name] = bsb
        convb_sb = ones_row_bf = None
        if convb is not None:
            convb_sb = singles.tile([1, D], bf16)
            nc.sync.dma_start(out=convb_sb[:], in_=convb.ap())
            ones_row_bf = singles.tile([1, 128], bf16)
            nc.vector.memset(ones_row_bf[:], 1.0)

        g2p = ctx.enter_context(tc.tile_pool(name="g2", bufs=2))
        g3p = ctx.enter_context(tc.tile_pool(name="g3", bufs=2))
        hstp = ctx.enter_context(tc.tile_pool(name="hstp", bufs=2))
        work = ctx.enter_context(tc.tile_pool(name="work", bufs=2))
        small = ctx.enter_context(tc.tile_pool(name="small", bufs=2))
        ypool = ctx.enter_context(tc.tile_pool(name="ypool", bufs=4))
        upool = ctx.enter_context(tc.tile_pool(name="upool", bufs=2))
        outsp = ctx.enter_context(tc.tile_pool(name="outs", bufs=2))
        psum_big = ctx.enter_context(tc.tile_pool(name="psb", bufs=3, space="PSUM"))
        psum_small = ctx.enter_context(tc.tile_pool(name="pss", bufs=2, space="PSUM"))
        psum_out = ctx.enter_context(tc.tile_pool(name="pso", bufs=2, space="PSUM"))

        st = {}  # per-tile state passed between pipeline stages

        def stage_a(i):
            """Gathers + hst load + We matmuls + e_t evac/square."""
            t0 = i * NT
            e2 = g2p.tile([128, DC, NT], bf16, tag="e2")
            nc.gpsimd.dma_gather(
                out_ap=e2[:],
                in_ap=emb2.ap(),
                idxs_ap=idx2_sb[:, i * (NT // 16) : (i + 1) * (NT // 16)],
                num_idxs=NT,
                num_idxs_reg=NT,
                elem_size=D,
                transpose=True,
            )
            e3r = g3p.tile([128, 2 * DC, NT], bf16, tag="e3r")
            nc.gpsimd.dma_gather(
                out_ap=e3r[:],
                in_ap=emb3p.ap(),
                idxs_ap=idx3_sb[:, i * (NT // 16) : (i + 1) * (NT // 16)],
                num_idxs=NT,
                num_idxs_reg=NT,
                elem_size=2 * D,
                transpose=True,
            )
            par_slice = par_sb[:, t0 : t0 + NT]
            for cc in range(DC):
                nc.vector.copy_predicated(
                    out=e3r[:, cc, :], mask=par_slice, data=e3r[:, DC + cc, :]
                )
            hst_t = hstp.tile([128, DC, NT], bf16, tag="hst")
            nc.sync.dma_start(out=hst_t[:], in_=hst_r[:, :, t0 : t0 + NT])

            et = work.tile([128, DC, NT], bf16, tag="et")
            et2 = work.tile([128, DC, NT], bf16, tag="et2")
            for m in range(DC):
                pet = psum_big.tile([128, NT], f32, tag="pbig")
                for k in range(KC):
                    rhs = e2[:, k, :] if k < DC else e3r[:, k - DC, :]
                    nc.tensor.matmul(
                        pet[:],
                        wet_sb[:, k, m * 128 : (m + 1) * 128],
                        rhs,
                        start=(k == 0),
                        stop=(k == KC - 1 and web is None),
                    )
                if web is not None:
                    nc.tensor.matmul(
                        pet[:],
                        bias_sbs["web"][:, m * 128 : (m + 1) * 128],
                        ones_nt_bf[:],
                        start=False,
                        stop=True,
                    )
                nc.scalar.activation(et[:, m, :], pet[:], AF.Copy)
                nc.vector.tensor_mul(et2[:, m, :], et[:, m, :], et[:, m, :])
            st[i] = (et, et2, hst_t)

        def stage_b(i):
            """RMS scale chain, Wk + dot + alpha, Wv + y."""
            t0 = i * NT
            et, et2, hst_t = st.pop(i)
            pms = psum_small.tile([1, NT], f32, tag="psmall")
            for m in range(DC):
                nc.tensor.matmul(
                    pms[:],
                    ones_col_bf[:],
                    et2[:, m, :],
                    start=(m == 0),
                    stop=(m == DC - 1),
                )
            sq = small.tile([1, NT], f32, tag="sq")
            nc.scalar.activation(sq[:], pms[:], AF.Sqrt, bias=eps_sb[:], scale=1.0 / D)
            se = small.tile([1, NT], f32, tag="se")
            nc.vector.reciprocal(se[:], sq[:])
            psb_t = psum_small.tile([128, NT], f32, tag="psmall")
            nc.tensor.matmul(psb_t[:], ones_row_f[:], se[:], start=True, stop=True)
            sbf = work.tile([128, NT], bf16, tag="sbf")
            nc.scalar.activation(sbf[:], psb_t[:], AF.Copy)
            en = work.tile([128, DC, NT], bf16, tag="en")
            for m in range(DC):
                nc.vector.tensor_mul(en[:, m, :], et[:, m, :], sbf[:])
            prod = work.tile([128, DC, NT], bf16, tag="prod")
            for m in range(DC):
                pke = psum_big.tile([128, NT], f32, tag="pbig")
                for k in range(DC):
                    nc.tensor.matmul(
                        pke[:],
                        wkt_sb[:, k, m * 128 : (m + 1) * 128],
                        en[:, k, :],
                        start=(k == 0),
                        stop=(k == DC - 1 and wkb is None),
                    )
                if wkb is not None:
                    nc.tensor.matmul(
                        pke[:],
                        bias_sbs["wkb"][:, m * 128 : (m + 1) * 128],
                        ones_nt_bf[:],
                        start=False,
                        stop=True,
                    )
                nc.vector.tensor_mul(prod[:, m, :], pke[:], hst_t[:, m, :])
            pdot = psum_small.tile([1, NT], f32, tag="psmall")
            for m in range(DC):
                nc.tensor.matmul(
                    pdot[:],
                    ones_col_bf[:],
                    prod[:, m, :],
                    start=(m == 0),
                    stop=(m == DC - 1),
                )
            d2 = small.tile([1, NT], f32, tag="d2")
            nc.vector.tensor_mul(d2[:], pdot[:], shs_sb[:, t0 : t0 + NT])
            alph = small.tile([1, NT], f32, tag="alph")
            nc.scalar.activation(alph[:], d2[:], AF.Sigmoid)
            alphm = small.tile([1, NT], f32, tag="alphm")
            nc.vector.tensor_mul(alphm[:], alph[:], ymask_sb[:, t0 : t0 + NT])
            pab = psum_small.tile([128, NT], f32, tag="psmall")
            nc.tensor.matmul(pab[:], ones_row_f[:], alphm[:], start=True, stop=True)
            abf = work.tile([128, NT], bf16, tag="abf")
            nc.scalar.activation(abf[:], pab[:], AF.Copy)
            y_t = ypool.tile([128, DC, NT], bf16, tag="y")
            for m in range(DC):
                pve = psum_big.tile([128, NT], f32, tag="pbig")
                for k in range(DC):
                    nc.tensor.matmul(
                        pve[:],
                        wvt_sb[:, k, m * 128 : (m + 1) * 128],
                        et[:, k, :],
                        start=(k == 0),
                        stop=(k == DC - 1 and wvb is None),
                    )
                if wvb is not None:
                    nc.tensor.matmul(
                        pve[:],
                        bias_sbs["wvb"][:, m * 128 : (m + 1) * 128],
                        ones_nt_bf[:],
                        start=False,
                        stop=True,
                    )
                nc.vector.tensor_mul(y_t[:, m, :], pve[:], abf[:])
            st[("y", i)] = y_t

        def stage_c(i):
            """Conv + PE transpose + residual add + store for tile i's
            central output range (ext positions [max(128,i*NT), ...))."""
            o0 = max(HALO, i * NT)
            o1 = min(T_EXT - HALO, (i + 1) * NT)
            olen = o1 - o0
            if olen <= 0:
                return
            y_t = st[("y", i)]
            yl = st.get(("y", i - 1))
            yr = st.get(("y", i + 1))
            lo = o0 - i * NT  # offset of o0 within tile i (0 or HALO)
            u_t = upool.tile([128, DC, NT], bf16, tag="u")
            for c in range(DC):
                for j in range(3):
                    s = lo - 1 + j  # tap j reads y ext [o0-1+j, o0-1+j+olen)
                    srcs = []
                    if s < 0:
                        srcs.append((yl[:, c, NT + s : NT + s + 1], 0, 1))
                        srcs.append((y_t[:, c, 0 : s + olen], -s, s + olen))
                    elif s + olen > NT:
                        srcs.append((y_t[:, c, s:NT], 0, NT - s))
                        srcs.append(
                            (yr[:, c, 0 : s + olen - NT], NT - s, s + olen - NT)
                        )
                    else:
                        srcs.append((y_t[:, c, s : s + olen], 0, olen))
                    for src_ap, dsto, dlen in srcs:
                        if j == 0:
                            nc.vector.tensor_scalar(
                                out=u_t[:, c, dsto : dsto + dlen],
                                in0=src_ap,
                                scalar1=convw_sb[:, c, 0:1],
                                scalar2=None,
                                op0=ALU.mult,
                            )
                        else:
                            nc.vector.scalar_tensor_tensor(
                                out=u_t[:, c, dsto : dsto + dlen],
                                in0=src_ap,
                                scalar=convw_sb[:, c, j : j + 1],
                                in1=u_t[:, c, dsto : dsto + dlen],
                                op0=ALU.mult,
                                op1=ALU.add,
                            )
            g0 = o0 - HALO  # output token index of tile's first output
            for tt in range(olen // 128):
                pu = psum_out.tile([128, D], bf16, tag="pu")
                if convb is not None:
                    for half in range(2):
                        nc.tensor.matmul(
                            pu[:, half * 512 : (half + 1) * 512],
                            ones_row_bf[:],
                            convb_sb[:, half * 512 : (half + 1) * 512],
                            start=True,
                            stop=False,
                        )
                for c in range(DC):
                    nc.tensor.matmul(
                        pu[:, c * 128 : (c + 1) * 128],
                        u_t[:, c, tt * 128 : (tt + 1) * 128],
                        identity_bf[:],
                        is_transpose=True,
                        start=(convb is None),
                        stop=True,
                    )
                hs_t = outsp.tile([128, D], f32, tag="hs")
                nc.sync.dma_start(
                    out=hs_t[:],
                    in_=hsc.ap()[g0 + tt * 128 : g0 + (tt + 1) * 128, :],
                )
                ot = outsp.tile([128, D], f32, tag="ot")
                nc.vector.tensor_add(ot[:], pu[:], hs_t[:])
                nc.sync.dma_start(
                    out=outp.ap()[g0 + tt * 128 : g0 + (tt + 1) * 128, :],
                    in_=ot[:],
                )

        # ---- lag-1 software pipeline ----
        stage_a(0)
        stage_a(1)
        stage_b(0)
        for i in range(1, NTILES):
            if i + 1 < NTILES:
                stage_a(i + 1)
            stage_b(i)
            stage_c(i - 1)
        stage_c(NTILES - 1)

    nc.compile()
    return nc


def _get_program(flags):
    if flags not in _PROG_CACHE:
        _PROG_CACHE[flags] = _build_program(*flags)
    return _PROG_CACHE[flags]


def _host_prep(inputs):
    hs = np.asarray(inputs["hidden_states"], dtype=np.float32)
    ids = np.asarray(inputs["input_ids"], dtype=np.int64)
    vproj = np.asarray(inputs["vocab_projection"], dtype=np.int64)
    emb2 = np.asarray(inputs["emb2"], dtype=np.float32)
    emb3 = np.asarray(inputs["emb3"], dtype=np.float32)
    We_w = np.asarray(inputs["We_w"], dtype=np.float32)
    We_b = np.asarray(inputs["We_b"], dtype=np.float32)
    Wv_w = np.asarray(inputs["Wv_w"], dtype=np.float32)
    Wv_b = np.asarray(inputs["Wv_b"], dtype=np.float32)
    Wk_w = np.asarray(inputs["Wk_w"], dtype=np.float32)
    Wk_b = np.asarray(inputs["Wk_b"], dtype=np.float32)
    conv_w = np.asarray(inputs["conv_w"], dtype=np.float32)
    conv_b = np.asarray(inputs["conv_b"], dtype=np.float32)
    norm_w = np.asarray(inputs["norm_w"], dtype=np.float32)

    # exact integer hash indices (host, int64)
    comp = vproj[ids]  # [B, S]
    padded = np.pad(comp, ((0, 0), (2, 0)))
    bi = padded[:, 0:S] + padded[:, 1 : S + 1]
    tri = bi + padded[:, 2 : S + 2]
    idx2 = ((bi * MULT) % HASH2).reshape(-1)
    idx3 = ((tri * MULT) % HASH3).reshape(-1)

    hsf = hs.reshape(B * S, D)
    msh = np.mean(np.square(hsf.astype(np.float64)), axis=1)
    sh = (1.0 / (np.sqrt(msh + EPS) * np.sqrt(D))).astype(np.float32)  # [B*S]
    hsn = hsf * norm_w[None, :]

    shared = {
        "emb2": emb2.astype(BF16),
        "emb3p": emb3.astype(BF16).reshape(HASH3 // 2, 2 * D),
        "wet": np.ascontiguousarray(We_w.T).astype(BF16),
        "wkt": np.ascontiguousarray(norm_w[:, None] * Wk_w.T).astype(BF16),
        "wvt": np.ascontiguousarray(Wv_w.T).astype(BF16),
        "convw": np.ascontiguousarray(
            conv_w[:, 0, :].reshape(DC, 128, 3).transpose(1, 0, 2)
        ).astype(np.float32),
    }
    flags = (
        bool(np.any(We_b)),
        bool(np.any(Wk_b)),
        bool(np.any(Wv_b)),
        bool(np.any(conv_b)),
    )
    if flags[0]:
        shared["web"] = We_b.reshape(1, D).astype(BF16)
    if flags[1]:
        shared["wkb"] = Wk_b.reshape(1, D).astype(BF16)
    if flags[2]:
        shared["wvb"] = Wv_b.reshape(1, D).astype(BF16)
    if flags[3]:
        shared["convb"] = conv_b.reshape(1, D).astype(BF16)

    def wrap16(a):
        return np.ascontiguousarray(
            np.tile(a.astype(np.int16).reshape(T_EXT // 16, 16).T, (8, 1))
        )

    in_maps = []
    for c in range(N_CORES):
        s0 = c * T_CORE
        ext = np.arange(s0 - HALO, s0 + T_CORE + HALO)
        cl = np.clip(ext, 0, B * S - 1)
        row = s0 // S
        inrow = ((ext >= row * S) & (ext < (row + 1) * S)).astype(np.float32)
        i2e = idx2[cl]
        i3e = idx3[cl]
        m = dict(shared)
        m["idx2r"] = wrap16(i2e)
        m["idx3r"] = wrap16(i3e >> 1)
        m["parity"] = (i3e & 1).astype(np.uint8)[None, :]
        m["ymask"] = inrow.astype(BF16)[None, :]
        m["shs"] = np.ascontiguousarray(sh[cl][None, :])
        m["hst"] = np.ascontiguousarray(hsn[cl].T).astype(BF16)
        m["hsc"] = np.ascontiguousarray(hsf[s0 : s0 + T_CORE])
        in_maps.append(m)
    return flags, in_maps


def kernel(**inputs) -> np.ndarray:
    flags, in_maps = _host_prep(inputs)
    nc = _get_program(flags)
    res = run_bass_kernel_spmd(nc, in_maps, core_ids=list(range(N_CORES)))
    out = np.concatenate(
        [res.results[c]["outp"] for c in range(N_CORES)], axis=0
    ).reshape(B, S, D)
    return np.ascontiguousarray(out, dtype=np.float32)


# revision 9
# speedup vs baseline: 1.2579x; 1.0886x over previous
"""Trainium2 Bass kernel for nn_EngramMemory_81415400063490 (embedding_lookup).

Contract: kernel(**inputs) takes the FULL unsharded inputs (numpy arrays, keyed
as in reference.setup_inputs()) and returns the FULL [4, 4096, 1024] float32
output. Internally shards data-parallel over the 8 NeuronCores (2048 tokens per
core + 128-token halo each side for the depthwise conv), replicates the hash
embedding tables + weights, runs one SPMD Bass program via
run_bass_kernel_spmd, and reassembles.

Device dataflow per core (feature-major activations, bf16 matmuls):
  dma_gather(transpose=True) pulls emb2 rows and emb3 row-PAIRS (the pair
  trick keeps indices inside int16) straight into feature-major layout; a
  predicated copy selects the odd row where idx3 is odd. A lag-1 software
  pipeline overlaps tile i+1's gather + We matmuls with tile i's dependent
  chain (RMS scale, Wk/dot/sigmoid, Wv, y=alpha*v) and tile i-1's conv +
  PE-transpose + residual-add + store.
"""

import sys

sys.path.insert(0, "/opt/trn_rl_repo")

import numpy as np
import ml_dtypes

import concourse.bass as bass
import concourse.tile as tile
from concourse import bacc, mybir
from concourse.bass_utils import run_bass_kernel_spmd
from concourse.masks import make_identity

BF16 = ml_dtypes.bfloat16
AF = mybir.ActivationFunctionType
ALU = mybir.AluOpType

B, S, D = 4, 4096, 1024
VOCAB, HASH2, HASH3 = 50257, 10000, 50000
MULT = 2654435761
EPS = 1.1920928955078125e-07  # torch float32 eps, used by the RMSNorm
N_CORES = 8
T_CORE = (B * S) // N_CORES  # 2048 tokens per core
HALO = 128
T_EXT = T_CORE + 2 * HALO  # 2304 tokens incl. halos
NT = 256  # token tile size
NTILES = T_EXT // NT  # 9
DC = D // 128  # 8 feature chunks of 128
KC = (2 * D) // 128  # 16 contraction chunks for We

_PROG_CACHE = {}


def _build_program(with_web, with_wkb, with_wvb, with_convb):
    f32, bf16, i16 = mybir.dt.float32, mybir.dt.bfloat16, mybir.dt.int16
    nc = bacc.Bacc("TRN2", target_bir_lowering=False)

    emb2 = nc.dram_tensor("emb2", [HASH2, D], bf16, kind="ExternalInput")
    emb3p = nc.dram_tensor("emb3p", [HASH3 // 2, 2 * D], bf16, kind="ExternalInput")
    wet = nc.dram_tensor("wet", [2 * D, D], bf16, kind="ExternalInput")
    wkt = nc.dram_tensor("wkt", [D, D], bf16, kind="ExternalInput")
    wvt = nc.dram_tensor("wvt", [D, D], bf16, kind="ExternalInput")
    convw = nc.dram_tensor("convw", [128, DC, 3], f32, kind="ExternalInput")
    idx2r = nc.dram_tensor("idx2r", [128, T_EXT // 16], i16, kind="ExternalInput")
    idx3r = nc.dram_tensor("idx3r", [128, T_EXT // 16], i16, kind="ExternalInput")
    parity = nc.dram_tensor("parity", [1, T_EXT], mybir.dt.uint8, kind="ExternalInput")
    ymaskd = nc.dram_tensor("ymask", [1, T_EXT], bf16, kind="ExternalInput")
    shsd = nc.dram_tensor("shs", [1, T_EXT], f32, kind="ExternalInput")
    hst = nc.dram_tensor("hst", [D, T_EXT], bf16, kind="ExternalInput")
    hsc = nc.dram_tensor("hsc", [T_CORE, D], f32, kind="ExternalInput")
    outp = nc.dram_tensor("outp", [T_CORE, D], f32, kind="ExternalOutput")
    web = wkb = wvb = convb = None
    if with_web:
        web = nc.dram_tensor("web", [1, D], bf16, kind="ExternalInput")
    if with_wkb:
        wkb = nc.dram_tensor("wkb", [1, D], bf16, kind="ExternalInput")
    if with_wvb:
        wvb = nc.dram_tensor("wvb", [1, D], bf16, kind="ExternalInput")
    if with_convb:
        convb = nc.dram_tensor("convb", [1, D], bf16, kind="ExternalInput")

    hst_r = hst.ap().rearrange("(c p) t -> p c t", p=128)  # [128, 8, 2304]

    import contextlib

    with tile.TileContext(nc) as tc, contextlib.ExitStack() as ctx:
        singles = ctx.enter_context(tc.tile_pool(name="singles", bufs=1))
        idx2_sb = singles.tile([128, T_EXT // 16], i16)
        nc.sync.dma_start(out=idx2_sb[:], in_=idx2r.ap())
        idx3_sb = singles.tile([128, T_EXT // 16], i16)
        nc.sync.dma_start(out=idx3_sb[:], in_=idx3r.ap())
        par_sb = singles.tile([128, T_EXT], mybir.dt.uint8)
        par_bcast = bass.AP(
            tensor=parity.ap().tensor, offset=0, ap=[[0, 128], [1, T_EXT]]
        )
        nc.gpsimd.dma_start(out=par_sb[:], in_=par_bcast)
        # per-chunk weight tiles so matmuls only wait on the chunk they read
        wet_ch = []
        for k in range(KC):
            w = singles.tile([128, D], bf16, tag=f"wet{k}")
            nc.sync.dma_start(out=w[:], in_=wet.ap()[k * 128 : (k + 1) * 128, :])
            wet_ch.append(w)
        wkt_ch = []
        for k in range(DC):
            w = singles.tile([128, D], bf16, tag=f"wkt{k}")
            nc.sync.dma_start(out=w[:], in_=wkt.ap()[k * 128 : (k + 1) * 128, :])
            wkt_ch.append(w)
        wvt_ch = []
        for k in range(DC):
            w = singles.tile([128, D], bf16, tag=f"wvt{k}")
            nc.sync.dma_start(out=w[:], in_=wvt.ap()[k * 128 : (k + 1) * 128, :])
            wvt_ch.append(w)
        convw_sb = singles.tile([128, DC, 3], f32)
        nc.sync.dma_start(out=convw_sb[:], in_=convw.ap())
        ymask_sb = singles.tile([1, T_EXT], bf16)
        nc.sync.dma_start(out=ymask_sb[:], in_=ymaskd.ap())
        shs_sb = singles.tile([1, T_EXT], f32)
        nc.sync.dma_start(out=shs_sb[:], in_=shsd.ap())
        ones_col_bf = singles.tile([128, 1], bf16)
        nc.vector.memset(ones_col_bf[:], 1.0)
        ones_row_f = singles.tile([1, 128], f32)
        nc.vector.memset(ones_row_f[:], 1.0)
        identity_bf = singles.tile([128, 128], bf16)
        make_identity(nc, identity_bf[:])
        ones_nt_bf = singles.tile([1, NT], bf16)
        nc.vector.memset(ones_nt_bf[:], 1.0)
        eps_sb = singles.tile([1, 1], f32)
        nc.vector.memset(eps_sb[:], float(EPS))
        bias_sbs = {}
        for name, t in (("web", web), ("wkb", wkb), ("wvb", wvb)):
            if t is not None:
                bsb = singles.tile([1, D], bf16)
                nc.sync.dma_start(out=bsb[:], in_=t.ap())
                bias_sbs[name] = bsb
        convb_sb = ones_row_bf = None
        if convb is not None:
            convb_sb = singles.tile([1, D], bf16)
            nc.sync.dma_start(out=convb_sb[:], in_=convb.ap())
            ones_row_bf = singles.tile([1, 128], bf16)
            nc.vector.memset(ones_row_bf[:], 1.0)

        g2p = ctx.enter_context(tc.tile_pool(name="g2", bufs=3))
        g3p = ctx.enter_context(tc.tile_pool(name="g3", bufs=3))
        hstp = ctx.enter_context(tc.tile_pool(name="hstp", bufs=2))
        work = ctx.enter_context(tc.tile_pool(name="work", bufs=2))
        etp = ctx.enter_context(tc.tile_pool(name="etp", bufs=3))
        small = ctx.enter_context(tc.tile_pool(name="small", bufs=2))
        ypool = ctx.enter_context(tc.tile_pool(name="ypool", bufs=4))
        upool = ctx.enter_context(tc.tile_pool(name="upool", bufs=2))
        outsp = ctx.enter_context(tc.tile_pool(name="outs", bufs=2))
        psum_big = ctx.enter_context(tc.tile_pool(name="psb", bufs=4, space="PSUM"))
        psum_small = ctx.enter_context(tc.tile_pool(name="pss", bufs=2, space="PSUM"))
        psum_out = ctx.enter_context(tc.tile_pool(name="pso", bufs=2, space="PSUM"))

        st = {}  # per-tile state passed between pipeline stages

        def stage_gather(i):
            """Issue gathers + parity select for tile i (runs ~3 tiles ahead)."""
            t0 = i * NT
            e2 = g2p.tile([128, DC, NT], bf16, tag="e2")
            nc.gpsimd.dma_gather(
                out_ap=e2[:],
                in_ap=emb2.ap(),
                idxs_ap=idx2_sb[:, i * (NT // 16) : (i + 1) * (NT // 16)],
                num_idxs=NT,
                num_idxs_reg=NT,
                elem_size=D,
                transpose=True,
            )
            e3r = g3p.tile([128, 2 * DC, NT], bf16, tag="e3r")
            nc.gpsimd.dma_gather(
                out_ap=e3r[:],
                in_ap=emb3p.ap(),
                idxs_ap=idx3_sb[:, i * (NT // 16) : (i + 1) * (NT // 16)],
                num_idxs=NT,
                num_idxs_reg=NT,
                elem_size=2 * D,
                transpose=True,
            )
            par_slice = par_sb[:, t0 : t0 + NT]
            for cc in range(DC):
                nc.vector.copy_predicated(
                    out=e3r[:, cc, :], mask=par_slice, data=e3r[:, DC + cc, :]
                )
            st[("g", i)] = (e2, e3r)

        def stage_we(i):
            """We matmuls + e_t evac + square; also prefetch hst for tile i."""
            t0 = i * NT
            e2, e3r = st.pop(("g", i))
            hst_t = hstp.tile([128, DC, NT], bf16, tag="hst")
            nc.sync.dma_start(out=hst_t[:], in_=hst_r[:, :, t0 : t0 + NT])
            et = etp.tile([128, DC, NT], bf16, tag="et")
            et2 = work.tile([128, DC, NT], bf16, tag="et2")
            for m in range(DC):
                pet = psum_big.tile([128, NT], f32, tag="pbig")
                for k in range(KC):
                    rhs = e2[:, k, :] if k < DC else e3r[:, k - DC, :]
                    nc.tensor.matmul(
                        pet[:],
                        wet_ch[k][:, m * 128 : (m + 1) * 128],
                        rhs,
                        start=(k == 0),
                        stop=(k == KC - 1 and web is None),
                    )
                if web is not None:
                    nc.tensor.matmul(
                        pet[:],
                        bias_sbs["web"][:, m * 128 : (m + 1) * 128],
                        ones_nt_bf[:],
                        start=False,
                        stop=True,
                    )
                nc.scalar.activation(et[:, m, :], pet[:], AF.Copy)
                nc.vector.tensor_mul(et2[:, m, :], et[:, m, :], et[:, m, :])
            st[i] = (et, et2, hst_t)

        def stage_ms(i):
            """Mean-square partition-reduce + rsqrt for tile i."""
            et, et2, hst_t = st[i]
            pms = psum_small.tile([1, NT], f32, tag="psmall")
            for m in range(DC):
                nc.tensor.matmul(
                    pms[:],
                    ones_col_bf[:],
                    et2[:, m, :],
                    start=(m == 0),
                    stop=(m == DC - 1),
                )
            sq = small.tile([1, NT], f32, tag="sq")
            nc.scalar.activation(sq[:], pms[:], AF.Sqrt, bias=eps_sb[:], scale=1.0 / D)
            se = small.tile([1, NT], f32, tag="se")
            nc.vector.reciprocal(se[:], sq[:])
            st[("se", i)] = se

        def stage_norm(i):
            """Broadcast 1/rms and scale e_t -> e_norm."""
            et, et2, hst_t = st[i]
            se = st.pop(("se", i))
            psb_t = psum_small.tile([128, NT], f32, tag="psmall")
            nc.tensor.matmul(psb_t[:], ones_row_f[:], se[:], start=True, stop=True)
            sbf = work.tile([128, NT], bf16, tag="sbf")
            nc.scalar.activation(sbf[:], psb_t[:], AF.Copy)
            en = work.tile([128, DC, NT], bf16, tag="en")
            for m in range(DC):
                nc.vector.tensor_mul(en[:, m, :], et[:, m, :], sbf[:])
            st[("en", i)] = en

        def stage_wk(i):
            """Wk matmuls + h*k_e products."""
            et, et2, hst_t = st[i]
            en = st.pop(("en", i))
            prod = work.tile([128, DC, NT], bf16, tag="et2")
            for m in range(DC):
                pke = psum_big.tile([128, NT], f32, tag="pbig")
                for k in range(DC):
                    nc.tensor.matmul(
                        pke[:],
                        wkt_ch[k][:, m * 128 : (m + 1) * 128],
                        en[:, k, :],
                        start=(k == 0),
                        stop=(k == DC - 1 and wkb is None),
                    )
                if wkb is not None:
                    nc.tensor.matmul(
                        pke[:],
                        bias_sbs["wkb"][:, m * 128 : (m + 1) * 128],
                        ones_nt_bf[:],
                        start=False,
                        stop=True,
                    )
                nc.vector.tensor_mul(prod[:, m, :], pke[:], hst_t[:, m, :])
            st[("prod", i)] = prod

        def stage_dot(i):
            """Reduce products to logits, sigmoid -> masked alpha."""
            t0 = i * NT
            prod = st.pop(("prod", i))
            pdot = psum_small.tile([1, NT], f32, tag="psmall")
            for m in range(DC):
                nc.tensor.matmul(
                    pdot[:],
                    ones_col_bf[:],
                    prod[:, m, :],
                    start=(m == 0),
                    stop=(m == DC - 1),
                )
            d2 = small.tile([1, NT], f32, tag="d2")
            nc.vector.tensor_mul(d2[:], pdot[:], shs_sb[:, t0 : t0 + NT])
            alph = small.tile([1, NT], f32, tag="alph")
            nc.scalar.activation(alph[:], d2[:], AF.Sigmoid)
            alphm = small.tile([1, NT], f32, tag="alphm")
            nc.vector.tensor_mul(alphm[:], alph[:], ymask_sb[:, t0 : t0 + NT])
            st[("am", i)] = alphm

        def stage_abf(i):
            """Broadcast alpha across partitions (runs after We of i+1)."""
            alphm = st.pop(("am", i))
            pab = psum_small.tile([128, NT], f32, tag="psmall")
            nc.tensor.matmul(pab[:], ones_row_f[:], alphm[:], start=True, stop=True)
            abf = work.tile([128, NT], bf16, tag="abf")
            nc.scalar.activation(abf[:], pab[:], AF.Copy)
            st[("abf", i)] = abf

        def stage_wv(i):
            """Wv matmuls + y = alpha * v_e."""
            et, et2, hst_t = st.pop(i)
            abf = st.pop(("abf", i))
            y_t = ypool.tile([128, DC, NT], bf16, tag="y")
            for m in range(DC):
                pve = psum_big.tile([128, NT], f32, tag="pbig")
                for k in range(DC):
                    nc.tensor.matmul(
                        pve[:],
                        wvt_ch[k][:, m * 128 : (m + 1) * 128],
                        et[:, k, :],
                        start=(k == 0),
                        stop=(k == DC - 1 and wvb is None),
                    )
                if wvb is not None:
                    nc.tensor.matmul(
                        pve[:],
                        bias_sbs["wvb"][:, m * 128 : (m + 1) * 128],
                        ones_nt_bf[:],
                        start=False,
                        stop=True,
                    )
                nc.vector.tensor_mul(y_t[:, m, :], pve[:], abf[:])
            st[("y", i)] = y_t

        def stage_conv(i):
            """Depthwise conv into u for tile i's central output range."""
            o0 = max(HALO, i * NT)
            o1 = min(T_EXT - HALO, (i + 1) * NT)
            olen = o1 - o0
            if olen <= 0:
                return
            y_t = st[("y", i)]
            yl = st.get(("y", i - 1))
            yr = st.get(("y", i + 1))
            lo = o0 - i * NT
            u_t = upool.tile([128, DC, NT], bf16, tag="u")
            for c in range(DC):
                for j in range(3):
                    s = lo - 1 + j
                    srcs = []
                    if s < 0:
                        srcs.append((yl[:, c, NT + s : NT + s + 1], 0, 1))
                        srcs.append((y_t[:, c, 0 : s + olen], -s, s + olen))
                    elif s + olen > NT:
                        srcs.append((y_t[:, c, s:NT], 0, NT - s))
                        srcs.append(
                            (yr[:, c, 0 : s + olen - NT], NT - s, s + olen - NT)
                        )
                    else:
                        srcs.append((y_t[:, c, s : s + olen], 0, olen))
                    for src_ap, dsto, dlen in srcs:
                        if j == 0:
                            nc.vector.tensor_scalar(
                                out=u_t[:, c, dsto : dsto + dlen],
                                in0=src_ap,
                                scalar1=convw_sb[:, c, 0:1],
                                scalar2=None,
                                op0=ALU.mult,
                            )
                        else:
                            nc.vector.scalar_tensor_tensor(
                                out=u_t[:, c, dsto : dsto + dlen],
                                in0=src_ap,
                                scalar=convw_sb[:, c, j : j + 1],
                                in1=u_t[:, c, dsto : dsto + dlen],
                                op0=ALU.mult,
                                op1=ALU.add,
                            )
            st[("u", i)] = (u_t, o0, olen)

        def stage_out(i):
            """PE transpose + residual add + store for tile i."""
            if ("u", i) not in st:
                return
            u_t, o0, olen = st.pop(("u", i))
            g0 = o0 - HALO
            for tt in range(olen // 128):
                pu = psum_out.tile([128, D], bf16, tag="pu")
                if convb is not None:
                    for half in range(2):
                        nc.tensor.matmul(
                            pu[:, half * 512 : (half + 1) * 512],
                            ones_row_bf[:],
                            convb_sb[:, half * 512 : (half + 1) * 512],
                            start=True,
                            stop=False,
                        )
                for c in range(DC):
                    nc.tensor.matmul(
                        pu[:, c * 128 : (c + 1) * 128],
                        u_t[:, c, tt * 128 : (tt + 1) * 128],
                        identity_bf[:],
                        is_transpose=True,
                        start=(convb is None),
                        stop=True,
                    )
                hs_t = outsp.tile([128, D], f32, tag="hs")
                nc.sync.dma_start(
                    out=hs_t[:],
                    in_=hsc.ap()[g0 + tt * 128 : g0 + (tt + 1) * 128, :],
                )
                ot = outsp.tile([128, D], f32, tag="ot")
                nc.vector.tensor_add(ot[:], pu[:], hs_t[:])
                nc.sync.dma_start(
                    out=outp.ap()[g0 + tt * 128 : g0 + (tt + 1) * 128, :],
                    in_=ot[:],
                )

        # ---- software pipeline ----
        # steady-state PE stream per iteration i:
        #   ms(i) | Wv(i-1)+y | bcast(i) | transposes(i-2) | Wk(i) | dot(i)
        #   | We(i+1) | alpha-bcast(i)
        stage_gather(0)
        stage_gather(1)
        stage_gather(2)
        stage_we(0)
        for i in range(NTILES):
            stage_ms(i)
            if i >= 1:
                stage_wv(i - 1)
            stage_norm(i)
            if i >= 2:
                stage_conv(i - 2)
                stage_out(i - 2)
            stage_wk(i)
            stage_dot(i)
            if i + 1 < NTILES:
                stage_we(i + 1)
            stage_abf(i)
            if i + 3 < NTILES:
                stage_gather(i + 3)
        stage_wv(NTILES - 1)
        stage_conv(NTILES - 2)
        stage_out(NTILES - 2)
        stage_conv(NTILES - 1)
        stage_out(NTILES - 1)

    nc.compile()
    return nc


def _get_program(flags):
    if flags not in _PROG_CACHE:
        _PROG_CACHE[flags] = _build_program(*flags)
    return _PROG_CACHE[flags]


def _host_prep(inputs):
    hs = np.asarray(inputs["hidden_states"], dtype=np.float32)
    ids = np.asarray(inputs["input_ids"], dtype=np.int64)
    vproj = np.asarray(inputs["vocab_projection"], dtype=np.int64)
    emb2 = np.asarray(inputs["emb2"], dtype=np.float32)
    emb3 = np.asarray(inputs["emb3"], dtype=np.float32)
    We_w = np.asarray(inputs["We_w"], dtype=np.float32)
    We_b = np.asarray(inputs["We_b"], dtype=np.float32)
    Wv_w = np.asarray(inputs["Wv_w"], dtype=np.float32)
    Wv_b = np.asarray(inputs["Wv_b"], dtype=np.float32)
    Wk_w = np.asarray(inputs["Wk_w"], dtype=np.float32)
    Wk_b = np.asarray(inputs["Wk_b"], dtype=np.float32)
    conv_w = np.asarray(inputs["conv_w"], dtype=np.float32)
    conv_b = np.asarray(inputs["conv_b"], dtype=np.float32)
    norm_w = np.asarray(inputs["norm_w"], dtype=np.float32)

    # exact integer hash indices (host, int64)
    comp = vproj[ids]  # [B, S]
    padded = np.pad(comp, ((0, 0), (2, 0)))
    bi = padded[:, 0:S] + padded[:, 1 : S + 1]
    tri = bi + padded[:, 2 : S + 2]
    idx2 = ((bi * MULT) % HASH2).reshape(-1)
    idx3 = ((tri * MULT) % HASH3).reshape(-1)

    hsf = hs.reshape(B * S, D)
    msh = np.mean(np.square(hsf.astype(np.float64)), axis=1)
    sh = (1.0 / (np.sqrt(msh + EPS) * np.sqrt(D))).astype(np.float32)  # [B*S]
    hsn = hsf * norm_w[None, :]

    shared = {
        "emb2": emb2.astype(BF16),
        "emb3p": emb3.astype(BF16).reshape(HASH3 // 2, 2 * D),
        "wet": np.ascontiguousarray(We_w.T).astype(BF16),
        "wkt": np.ascontiguousarray(norm_w[:, None] * Wk_w.T).astype(BF16),
        "wvt": np.ascontiguousarray(Wv_w.T).astype(BF16),
        "convw": np.ascontiguousarray(
            conv_w[:, 0, :].reshape(DC, 128, 3).transpose(1, 0, 2)
        ).astype(np.float32),
    }
    flags = (
        bool(np.any(We_b)),
        bool(np.any(Wk_b)),
        bool(np.any(Wv_b)),
        bool(np.any(conv_b)),
    )
    if flags[0]:
        shared["web"] = We_b.reshape(1, D).astype(BF16)
    if flags[1]:
        shared["wkb"] = Wk_b.reshape(1, D).astype(BF16)
    if flags[2]:
        shared["wvb"] = Wv_b.reshape(1, D).astype(BF16)
    if flags[3]:
        shared["convb"] = conv_b.reshape(1, D).astype(BF16)

    def wrap16(a):
        return np.ascontiguousarray(
            np.tile(a.astype(np.int16).reshape(T_EXT // 16, 16).T, (8, 1))
        )

    in_maps = []
    for c in range(N_CORES):
        s0 = c * T_CORE
        ext = np.arange(s0 - HALO, s0 + T_CORE + HALO)
        cl = np.clip(ext, 0, B * S - 1)
        row = s0 // S
        inrow = ((ext >= row * S) & (ext < (row + 1) * S)).astype(np.float32)
        i2e = idx2[cl]
        i3e = idx3[cl]
        m = dict(shared)
        m["idx2r"] = wrap16(i2e)
        m["idx3r"] = wrap16(i3e >> 1)
        m["parity"] = (i3e & 1).astype(np.uint8)[None, :]
        m["ymask"] = inrow.astype(BF16)[None, :]
        m["shs"] = np.ascontiguousarray(sh[cl][None, :])
        m["hst"] = np.ascontiguousarray(hsn[cl].T).astype(BF16)
        m["hsc"] = np.ascontiguousarray(hsf[s0 : s0 + T_CORE])
        in_maps.append(m)
    return flags, in_maps


def kernel(**inputs) -> np.ndarray:
    flags, in_maps = _host_prep(inputs)
    nc = _get_program(flags)
    res = run_bass_kernel_spmd(nc, in_maps, core_ids=list(range(N_CORES)))
    out = np.concatenate(
        [res.results[c]["outp"] for c in range(N_CORES)], axis=0
    ).reshape(B, S, D)
    return np.ascontiguousarray(out, dtype=np.float32)


# revision 14
# speedup vs baseline: 1.2822x; 1.0193x over previous
"""Trainium2 Bass kernel for nn_EngramMemory_81415400063490 (embedding_lookup).

Contract: kernel(**inputs) takes the FULL unsharded inputs (numpy arrays, keyed
as in reference.setup_inputs()) and returns the FULL [4, 4096, 1024] float32
output. Internally shards data-parallel over the 8 NeuronCores (2048 tokens per
core + 128-token halo each side for the depthwise conv), replicates the hash
embedding tables + weights, runs one SPMD Bass program via
run_bass_kernel_spmd, and reassembles.

Device dataflow per core (feature-major activations, bf16 matmuls):
  dma_gather(transpose=True) pulls emb2 rows and emb3 row-PAIRS (the pair
  trick keeps indices inside int16) straight into feature-major layout; a
  predicated copy selects the odd row where idx3 is odd. A lag-1 software
  pipeline overlaps tile i+1's gather + We matmuls with tile i's dependent
  chain (RMS scale, Wk/dot/sigmoid, Wv, y=alpha*v) and tile i-1's conv +
  PE-transpose + residual-add + store.
"""

import sys

sys.path.insert(0, "/opt/trn_rl_repo")

import numpy as np
import ml_dtypes

import concourse.bass as bass
import concourse.tile as tile
from concourse import bacc, mybir
from concourse.bass_utils import run_bass_kernel_spmd
from concourse.masks import make_identity

BF16 = ml_dtypes.bfloat16
AF = mybir.ActivationFunctionType
ALU = mybir.AluOpType

B, S, D = 4, 4096, 1024
VOCAB, HASH2, HASH3 = 50257, 10000, 50000
MULT = 2654435761
EPS = 1.1920928955078125e-07  # torch float32 eps, used by the RMSNorm
N_CORES = 8
T_CORE = (B * S) // N_CORES  # 2048 tokens per core
HALO = 128
T_EXT = T_CORE + 2 * HALO  # 2304 tokens incl. halos
NT = 256  # token tile size
NTILES = T_EXT // NT  # 9
DC = D // 128  # 8 feature chunks of 128
KC = (2 * D) // 128  # 16 contraction chunks for We

_PROG_CACHE = {}


def _build_program(with_web, with_wkb, with_wvb, with_convb):
    f32, bf16, i16 = mybir.dt.float32, mybir.dt.bfloat16, mybir.dt.int16
    nc = bacc.Bacc("TRN2", target_bir_lowering=False)

    emb2 = nc.dram_tensor("emb2", [HASH2, D], bf16, kind="ExternalInput")
    emb3p = nc.dram_tensor("emb3p", [HASH3 // 2, 2 * D], bf16, kind="ExternalInput")
    wet = nc.dram_tensor("wet", [2 * D, D], bf16, kind="ExternalInput")
    wkt = nc.dram_tensor("wkt", [D, D], bf16, kind="ExternalInput")
    wvt = nc.dram_tensor("wvt", [D, D], bf16, kind="ExternalInput")
    convw = nc.dram_tensor("convw", [128, DC, 3], f32, kind="ExternalInput")
    idx2r = nc.dram_tensor("idx2r", [128, T_EXT // 16], i16, kind="ExternalInput")
    idx3r = nc.dram_tensor("idx3r", [128, T_EXT // 16], i16, kind="ExternalInput")
    parity = nc.dram_tensor("parity", [1, T_EXT], mybir.dt.uint8, kind="ExternalInput")
    ymaskd = nc.dram_tensor("ymask", [1, T_EXT], bf16, kind="ExternalInput")
    shsd = nc.dram_tensor("shs", [1, T_EXT], f32, kind="ExternalInput")
    hst = nc.dram_tensor("hst", [D, T_EXT], bf16, kind="ExternalInput")
    hsc = nc.dram_tensor("hsc", [T_CORE, D], f32, kind="ExternalInput")
    outp = nc.dram_tensor("outp", [T_CORE, D], f32, kind="ExternalOutput")
    web = wkb = wvb = convb = None
    if with_web:
        web = nc.dram_tensor("web", [1, D], bf16, kind="ExternalInput")
    if with_wkb:
        wkb = nc.dram_tensor("wkb", [1, D], bf16, kind="ExternalInput")
    if with_wvb:
        wvb = nc.dram_tensor("wvb", [1, D], bf16, kind="ExternalInput")
    if with_convb:
        convb = nc.dram_tensor("convb", [1, D], bf16, kind="ExternalInput")

    hst_r = hst.ap().rearrange("(c p) t -> p c t", p=128)  # [128, 8, 2304]

    import contextlib

    with tile.TileContext(nc) as tc, contextlib.ExitStack() as ctx:
        singles = ctx.enter_context(tc.tile_pool(name="singles", bufs=1))
        idx2_sb = singles.tile([128, T_EXT // 16], i16)
        nc.sync.dma_start(out=idx2_sb[:], in_=idx2r.ap())
        idx3_sb = singles.tile([128, T_EXT // 16], i16)
        nc.sync.dma_start(out=idx3_sb[:], in_=idx3r.ap())
        par_sb = singles.tile([128, T_EXT], mybir.dt.uint8)
        par_bcast = bass.AP(
            tensor=parity.ap().tensor, offset=0, ap=[[0, 128], [1, T_EXT]]
        )
        nc.gpsimd.dma_start(out=par_sb[:], in_=par_bcast)
        # per-chunk weight tiles so matmuls only wait on the chunk they read;
        # DMAs are emitted AFTER the first gathers (see _load_weights below)
        wet_ch = [singles.tile([128, D], bf16, tag=f"wet{k}", name=f"wet{k}") for k in range(KC)]
        wkt_ch = [singles.tile([128, D], bf16, tag=f"wkt{k}", name=f"wkt{k}") for k in range(DC)]
        wvt_ch = [singles.tile([128, D], bf16, tag=f"wvt{k}", name=f"wvt{k}") for k in range(DC)]
        convw_sb = singles.tile([128, DC, 3], f32)

        def _load_weights():
            for k in range(KC):
                nc.sync.dma_start(
                    out=wet_ch[k][:], in_=wet.ap()[k * 128 : (k + 1) * 128, :]
                )
            for k in range(DC):
                nc.sync.dma_start(
                    out=wkt_ch[k][:], in_=wkt.ap()[k * 128 : (k + 1) * 128, :]
                )
            for k in range(DC):
                nc.sync.dma_start(
                    out=wvt_ch[k][:], in_=wvt.ap()[k * 128 : (k + 1) * 128, :]
                )
            nc.sync.dma_start(out=convw_sb[:], in_=convw.ap())
        ymask_sb = singles.tile([1, T_EXT], bf16)
        nc.sync.dma_start(out=ymask_sb[:], in_=ymaskd.ap())
        shs_sb = singles.tile([1, T_EXT], f32)
        nc.sync.dma_start(out=shs_sb[:], in_=shsd.ap())
        ones_col_bf = singles.tile([128, 1], bf16)
        nc.vector.memset(ones_col_bf[:], 1.0)
        ones_row_f = singles.tile([1, 128], f32)
        nc.vector.memset(ones_row_f[:], 1.0)
        identity_bf = singles.tile([128, 128], bf16)
        make_identity(nc, identity_bf[:])
        ones_nt_bf = singles.tile([1, NT], bf16)
        nc.vector.memset(ones_nt_bf[:], 1.0)
        eps_sb = singles.tile([1, 1], f32)
        nc.vector.memset(eps_sb[:], float(EPS))
        bias_sbs = {}
        for name, t in (("web", web), ("wkb", wkb), ("wvb", wvb)):
            if t is not None:
                bsb = singles.tile([1, D], bf16)
                nc.sync.dma_start(out=bsb[:], in_=t.ap())
                bias_sbs[name] = bsb
        convb_sb = ones_row_bf = None
        if convb is not None:
            convb_sb = singles.tile([1, D], bf16)
            nc.sync.dma_start(out=convb_sb[:], in_=convb.ap())
            ones_row_bf = singles.tile([1, 128], bf16)
            nc.vector.memset(ones_row_bf[:], 1.0)

        g2p = ctx.enter_context(tc.tile_pool(name="g2", bufs=4))
        g3p = ctx.enter_context(tc.tile_pool(name="g3", bufs=4))
        hstp = ctx.enter_context(tc.tile_pool(name="hstp", bufs=2))
        work = ctx.enter_context(tc.tile_pool(name="work", bufs=2))
        etp = ctx.enter_context(tc.tile_pool(name="etp", bufs=3))
        small = ctx.enter_context(tc.tile_pool(name="small", bufs=2))
        ypool = ctx.enter_context(tc.tile_pool(name="ypool", bufs=4))
        upool = ctx.enter_context(tc.tile_pool(name="upool", bufs=2))
        outsp = ctx.enter_context(tc.tile_pool(name="outs", bufs=2))
        psum_big = ctx.enter_context(tc.tile_pool(name="psb", bufs=4, space="PSUM"))
        psum_small = ctx.enter_context(tc.tile_pool(name="pss", bufs=2, space="PSUM"))
        psum_out = ctx.enter_context(tc.tile_pool(name="pso", bufs=2, space="PSUM"))

        st = {}  # per-tile state passed between pipeline stages

        def stage_gather(i):
            """Issue gathers + parity select for tile i (runs ~3 tiles ahead)."""
            t0 = i * NT
            e2 = g2p.tile([128, DC, NT], bf16, tag="e2")
            nc.gpsimd.dma_gather(
                out_ap=e2[:],
                in_ap=emb2.ap(),
                idxs_ap=idx2_sb[:, i * (NT // 16) : (i + 1) * (NT // 16)],
                num_idxs=NT,
                num_idxs_reg=NT,
                elem_size=D,
                transpose=True,
            )
            e3r = g3p.tile([128, 2 * DC, NT], bf16, tag="e3r")
            nc.gpsimd.dma_gather(
                out_ap=e3r[:],
                in_ap=emb3p.ap(),
                idxs_ap=idx3_sb[:, i * (NT // 16) : (i + 1) * (NT // 16)],
                num_idxs=NT,
                num_idxs_reg=NT,
                elem_size=2 * D,
                transpose=True,
            )
            par_slice = par_sb[:, t0 : t0 + NT]
            for cc in range(DC):
                nc.vector.copy_predicated(
                    out=e3r[:, cc, :], mask=par_slice, data=e3r[:, DC + cc, :]
                )
            st[("g", i)] = (e2, e3r)

        def stage_we(i):
            """We matmuls + e_t evac + square; also prefetch hst for tile i."""
            t0 = i * NT
            e2, e3r = st.pop(("g", i))
            hst_t = hstp.tile([128, DC, NT], bf16, tag="hst")
            nc.sync.dma_start(out=hst_t[:], in_=hst_r[:, :, t0 : t0 + NT])
            et = etp.tile([128, DC, NT], bf16, tag="et")
            et2 = work.tile([128, DC, NT], bf16, tag="et2")
            for m in range(DC):
                pet = psum_big.tile([128, NT], f32, tag="pbig")
                for k in range(KC):
                    rhs = e2[:, k, :] if k < DC else e3r[:, k - DC, :]
                    nc.tensor.matmul(
                        pet[:],
                        wet_ch[k][:, m * 128 : (m + 1) * 128],
                        rhs,
                        start=(k == 0),
                        stop=(k == KC - 1 and web is None),
                    )
                if web is not None:
                    nc.tensor.matmul(
                        pet[:],
                        bias_sbs["web"][:, m * 128 : (m + 1) * 128],
                        ones_nt_bf[:],
                        start=False,
                        stop=True,
                    )
                nc.scalar.activation(et[:, m, :], pet[:], AF.Copy)
                nc.vector.tensor_mul(et2[:, m, :], et[:, m, :], et[:, m, :])
            st[i] = (et, et2, hst_t)

        def stage_ms(i):
            """Mean-square partition-reduce + rsqrt for tile i."""
            et, et2, hst_t = st[i]
            pms = psum_small.tile([1, NT], f32, tag="psmall")
            for m in range(DC):
                nc.tensor.matmul(
                    pms[:],
                    ones_col_bf[:],
                    et2[:, m, :],
                    start=(m == 0),
                    stop=(m == DC - 1),
                )
            sq = small.tile([1, NT], f32, tag="tmp1")
            nc.scalar.activation(sq[:], pms[:], AF.Sqrt, bias=eps_sb[:], scale=1.0 / D)
            se = small.tile([1, NT], f32, tag="se")
            nc.vector.reciprocal(se[:], sq[:])
            st[("se", i)] = se

        def stage_norm(i):
            """Broadcast 1/rms and scale e_t -> e_norm."""
            et, et2, hst_t = st[i]
            se = st.pop(("se", i))
            psb_t = psum_small.tile([128, NT], f32, tag="psmall")
            nc.tensor.matmul(psb_t[:], ones_row_f[:], se[:], start=True, stop=True)
            sbf = work.tile([128, NT], bf16, tag="sbf")
            nc.scalar.activation(sbf[:], psb_t[:], AF.Copy)
            en = work.tile([128, DC, NT], bf16, tag="en")
            for m in range(DC):
                nc.vector.tensor_mul(en[:, m, :], et[:, m, :], sbf[:])
            st[("en", i)] = en

        def stage_wk(i):
            """Wk matmuls + h*k_e products."""
            et, et2, hst_t = st[i]
            en = st.pop(("en", i))
            prod = work.tile([128, DC, NT], bf16, tag="et2")
            for m in range(DC):
                pke = psum_big.tile([128, NT], f32, tag="pbig")
                for k in range(DC):
                    nc.tensor.matmul(
                        pke[:],
                        wkt_ch[k][:, m * 128 : (m + 1) * 128],
                        en[:, k, :],
                        start=(k == 0),
                        stop=(k == DC - 1 and wkb is None),
                    )
                if wkb is not None:
                    nc.tensor.matmul(
                        pke[:],
                        bias_sbs["wkb"][:, m * 128 : (m + 1) * 128],
                        ones_nt_bf[:],
                        start=False,
                        stop=True,
                    )
                nc.vector.tensor_mul(prod[:, m, :], pke[:], hst_t[:, m, :])
            st[("prod", i)] = prod

        def stage_dot(i):
            """Reduce products to logits, sigmoid -> masked alpha."""
            t0 = i * NT
            prod = st.pop(("prod", i))
            pdot = psum_small.tile([1, NT], f32, tag="psmall")
            for m in range(DC):
                nc.tensor.matmul(
                    pdot[:],
                    ones_col_bf[:],
                    prod[:, m, :],
                    start=(m == 0),
                    stop=(m == DC - 1),
                )
            d2 = small.tile([1, NT], f32, tag="tmp1")
            nc.vector.tensor_mul(d2[:], pdot[:], shs_sb[:, t0 : t0 + NT])
            alph = small.tile([1, NT], f32, tag="tmp1")
            nc.scalar.activation(alph[:], d2[:], AF.Sigmoid)
            alphm = small.tile([1, NT], f32, tag="tmp1")
            nc.vector.tensor_mul(alphm[:], alph[:], ymask_sb[:, t0 : t0 + NT])
            st[("am", i)] = alphm

        def stage_abf(i):
            """Broadcast alpha across partitions (runs after We of i+1)."""
            alphm = st.pop(("am", i))
            pab = psum_small.tile([128, NT], f32, tag="psmall")
            nc.tensor.matmul(pab[:], ones_row_f[:], alphm[:], start=True, stop=True)
            abf = work.tile([128, NT], bf16, tag="abf")
            nc.scalar.activation(abf[:], pab[:], AF.Copy)
            st[("abf", i)] = abf

        def stage_wv(i):
            """Wv matmuls + y = alpha * v_e."""
            et, et2, hst_t = st.pop(i)
            abf = st.pop(("abf", i))
            y_t = ypool.tile([128, DC, NT], bf16, tag="y")
            for m in range(DC):
                pve = psum_big.tile([128, NT], f32, tag="pbig")
                for k in range(DC):
                    nc.tensor.matmul(
                        pve[:],
                        wvt_ch[k][:, m * 128 : (m + 1) * 128],
                        et[:, k, :],
                        start=(k == 0),
                        stop=(k == DC - 1 and wvb is None),
                    )
                if wvb is not None:
                    nc.tensor.matmul(
                        pve[:],
                        bias_sbs["wvb"][:, m * 128 : (m + 1) * 128],
                        ones_nt_bf[:],
                        start=False,
                        stop=True,
                    )
                nc.vector.tensor_mul(y_t[:, m, :], pve[:], abf[:])
            st[("y", i)] = y_t

        def stage_conv(i):
            """Depthwise conv into u for tile i's central output range."""
            o0 = max(HALO, i * NT)
            o1 = min(T_EXT - HALO, (i + 1) * NT)
            olen = o1 - o0
            if olen <= 0:
                return
            y_t = st[("y", i)]
            yl = st.get(("y", i - 1))
            yr = st.get(("y", i + 1))
            lo = o0 - i * NT
            u_t = upool.tile([128, DC, NT], bf16, tag="u")
            for c in range(DC):
                for j in range(3):
                    s = lo - 1 + j
                    srcs = []
                    if s < 0:
                        srcs.append((yl[:, c, NT + s : NT + s + 1], 0, 1))
                        srcs.append((y_t[:, c, 0 : s + olen], -s, s + olen))
                    elif s + olen > NT:
                        srcs.append((y_t[:, c, s:NT], 0, NT - s))
                        srcs.append(
                            (yr[:, c, 0 : s + olen - NT], NT - s, s + olen - NT)
                        )
                    else:
                        srcs.append((y_t[:, c, s : s + olen], 0, olen))
                    for src_ap, dsto, dlen in srcs:
                        if j == 0:
                            nc.vector.tensor_scalar(
                                out=u_t[:, c, dsto : dsto + dlen],
                                in0=src_ap,
                                scalar1=convw_sb[:, c, 0:1],
                                scalar2=None,
                                op0=ALU.mult,
                            )
                        else:
                            nc.vector.scalar_tensor_tensor(
                                out=u_t[:, c, dsto : dsto + dlen],
                                in0=src_ap,
                                scalar=convw_sb[:, c, j : j + 1],
                                in1=u_t[:, c, dsto : dsto + dlen],
                                op0=ALU.mult,
                                op1=ALU.add,
                            )
            st[("u", i)] = (u_t, o0, olen)

        def stage_out(i):
            """PE transpose + residual add + store for tile i."""
            if ("u", i) not in st:
                return
            u_t, o0, olen = st.pop(("u", i))
            g0 = o0 - HALO
            for tt in range(olen // 128):
                pu = psum_out.tile([128, D], bf16, tag="pu")
                if convb is not None:
                    for half in range(2):
                        nc.tensor.matmul(
                            pu[:, half * 512 : (half + 1) * 512],
                            ones_row_bf[:],
                            convb_sb[:, half * 512 : (half + 1) * 512],
                            start=True,
                            stop=False,
                        )
                for c in range(DC):
                    nc.tensor.matmul(
                        pu[:, c * 128 : (c + 1) * 128],
                        u_t[:, c, tt * 128 : (tt + 1) * 128],
                        identity_bf[:],
                        is_transpose=True,
                        start=(convb is None),
                        stop=True,
                    )
                hs_t = outsp.tile([128, D], f32, tag="hs")
                nc.sync.dma_start(
                    out=hs_t[:],
                    in_=hsc.ap()[g0 + tt * 128 : g0 + (tt + 1) * 128, :],
                )
                nc.vector.tensor_add(hs_t[:], pu[:], hs_t[:])
                nc.sync.dma_start(
                    out=outp.ap()[g0 + tt * 128 : g0 + (tt + 1) * 128, :],
                    in_=hs_t[:],
                )

        # ---- software pipeline ----
        # steady-state PE stream per iteration i:
        #   ms(i) | Wv(i-1)+y | bcast(i) | transposes(i-2) | Wk(i) | dot(i)
        #   | We(i+1) | alpha-bcast(i)
        stage_gather(0)
        stage_gather(1)
        stage_gather(2)
        _load_weights()
        stage_we(0)
        for i in range(NTILES):
            if i + 3 < NTILES:
                stage_gather(i + 3)
            stage_ms(i)
            if i >= 1:
                stage_wv(i - 1)
            stage_norm(i)
            if i >= 2:
                stage_conv(i - 2)
                stage_out(i - 2)
            stage_wk(i)
            stage_dot(i)
            if i + 1 < NTILES:
                stage_we(i + 1)
            stage_abf(i)
        stage_wv(NTILES - 1)
        stage_conv(NTILES - 2)
        stage_out(NTILES - 2)
        stage_conv(NTILES - 1)
        stage_out(NTILES - 1)

    nc.compile()
    return nc


def _get_program(flags):
    if flags not in _PROG_CACHE:
        _PROG_CACHE[flags] = _build_program(*flags)
    return _PROG_CACHE[flags]


def _host_prep(inputs):
    hs = np.asarray(inputs["hidden_states"], dtype=np.float32)
    ids = np.asarray(inputs["input_ids"], dtype=np.int64)
    vproj = np.asarray(inputs["vocab_projection"], dtype=np.int64)
    emb2 = np.asarray(inputs["emb2"], dtype=np.float32)
    emb3 = np.asarray(inputs["emb3"], dtype=np.float32)
    We_w = np.asarray(inputs["We_w"], dtype=np.float32)
    We_b = np.asarray(inputs["We_b"], dtype=np.float32)
    Wv_w = np.asarray(inputs["Wv_w"], dtype=np.float32)
    Wv_b = np.asarray(inputs["Wv_b"], dtype=np.float32)
    Wk_w = np.asarray(inputs["Wk_w"], dtype=np.float32)
    Wk_b = np.asarray(inputs["Wk_b"], dtype=np.float32)
    conv_w = np.asarray(inputs["conv_w"], dtype=np.float32)
    conv_b = np.asarray(inputs["conv_b"], dtype=np.float32)
    norm_w = np.asarray(inputs["norm_w"], dtype=np.float32)

    # exact integer hash indices (host, int64)
    comp = vproj[ids]  # [B, S]
    padded = np.pad(comp, ((0, 0), (2, 0)))
    bi = padded[:, 0:S] + padded[:, 1 : S + 1]
    tri = bi + padded[:, 2 : S + 2]
    idx2 = ((bi * MULT) % HASH2).reshape(-1)
    idx3 = ((tri * MULT) % HASH3).reshape(-1)

    hsf = hs.reshape(B * S, D)
    msh = np.mean(np.square(hsf.astype(np.float64)), axis=1)
    sh = (1.0 / (np.sqrt(msh + EPS) * np.sqrt(D))).astype(np.float32)  # [B*S]
    hsn = hsf * norm_w[None, :]

    shared = {
        "emb2": emb2.astype(BF16),
        "emb3p": emb3.astype(BF16).reshape(HASH3 // 2, 2 * D),
        "wet": np.ascontiguousarray(We_w.T).astype(BF16),
        "wkt": np.ascontiguousarray(norm_w[:, None] * Wk_w.T).astype(BF16),
        "wvt": np.ascontiguousarray(Wv_w.T).astype(BF16),
        "convw": np.ascontiguousarray(
            conv_w[:, 0, :].reshape(DC, 128, 3).transpose(1, 0, 2)
        ).astype(np.float32),
    }
    flags = (
        bool(np.any(We_b)),
        bool(np.any(Wk_b)),
        bool(np.any(Wv_b)),
        bool(np.any(conv_b)),
    )
    if flags[0]:
        shared["web"] = We_b.reshape(1, D).astype(BF16)
    if flags[1]:
        shared["wkb"] = Wk_b.reshape(1, D).astype(BF16)
    if flags[2]:
        shared["wvb"] = Wv_b.reshape(1, D).astype(BF16)
    if flags[3]:
        shared["convb"] = conv_b.reshape(1, D).astype(BF16)

    def wrap16(a):
        return np.ascontiguousarray(
            np.tile(a.astype(np.int16).reshape(T_EXT // 16, 16).T, (8, 1))
        )

    in_maps = []
    for c in range(N_CORES):
        s0 = c * T_CORE
        ext = np.arange(s0 - HALO, s0 + T_CORE + HALO)
        cl = np.clip(ext, 0, B * S - 1)
        row = s0 // S
        inrow = ((ext >= row * S) & (ext < (row + 1) * S)).astype(np.float32)
        i2e = idx2[cl]
        i3e = idx3[cl]
        m = dict(shared)
        m["idx2r"] = wrap16(i2e)
        m["idx3r"] = wrap16(i3e >> 1)
        m["parity"] = (i3e & 1).astype(np.uint8)[None, :]
        m["ymask"] = inrow.astype(BF16)[None, :]
        m["shs"] = np.ascontiguousarray(sh[cl][None, :])
        m["hst"] = np.ascontiguousarray(hsn[cl].T).astype(BF16)
        m["hsc"] = np.ascontiguousarray(hsf[s0 : s0 + T_CORE])
        in_maps.append(m)
    return flags, in_maps


def kernel(**inputs) -> np.ndarray:
    flags, in_maps = _host_prep(inputs)
    nc = _get_program(flags)
    res = run_bass_kernel_spmd(nc, in_maps, core_ids=list(range(N_CORES)))
    out = np.concatenate(
        [res.results[c]["outp"] for c in range(N_CORES)], axis=0
    ).reshape(B, S, D)
    return np.ascontiguousarray(out, dtype=np.float32)
